# revision 1
# baseline (speedup 1.0000x reference)
"""Trainium2 Bass kernel for nn_AttEncode (8-core data-parallel over batch).

Reference computation (B=64, T=2048, D=1024, C=1024, F=256, K=5):
    label_norm = l2_normalize(label_embed, axis=-1)          # [C, D]
    G          = einsum('btd,cd->btc', S, label_norm)        # [B, T, C]
    conv       = relu(conv1d_same(G, conv_w) + conv_b)       # [B, T, F]
    att_v      = max(conv, axis=-1)                          # [B, T]
    H          = einsum('btd,bt->bd', S, att_v)              # [B, D]

Key algebraic reduction: G is only consumed linearly by the conv, so fold the
label matmul into the conv weights once:
    W2[k, d, f] = sum_c label_norm[c, d] * conv_w[k, c, f]   # [K, D, F] tiny
    conv[t, f]  = sum_k sum_d S[t+k-2, d] * W2[k, d, f]
which removes the entire [B,T,C] einsum (45% of the FLOPs).

Per-core dataflow (8 batches/core, ~1.0 ms on silicon):
  - HWDGE-load S f32, convert to bf16 on ScalarE, then one xbar
    DMA-transpose per 128-token chunk gives S^T tiles in [d, t] layout
    (transpose needs a contiguous dest; DVE copies assemble 132-wide
    haloed per-chunk tiles for the SAME conv padding)
  - 40 bf16 matmuls per chunk accumulate conv[t0:t0+128, 0:256] in PSUM
    (stationary operand = S^T slice; PE streams ~2 GHz effective)
  - DVE reduce_max over F + relu/bf16-cast on GpSimd -> att column [128,1]
  - att feeds two skinny N=512 matmuls accumulating H[b, :] in PSUM,
    emitted 4 chunks behind the conv so the att chain never stalls PE
  - loads/transposes run 4 chunks ahead in a flat loop over all 128
    chunks, so neither the H lag nor the lookahead drains at batch
    boundaries
"""

import os
import numpy as np

B, T, D, C, F, K = 64, 2048, 1024, 1024, 256, 5
N_CORES = 8
B_CORE = B // N_CORES
EPS = 1e-12
TCH = T // 128  # t-chunks per batch
DC = D // 128   # d chunks
CC = C // 128   # c chunks

_CACHE = {}


def _build_nc(with_bias):
    import concourse.mybir as mybir
    import concourse.tile as tile
    from concourse import bacc

    fp32 = mybir.dt.float32
    bf16 = mybir.dt.bfloat16
    ALU = mybir.AluOpType

    nc = bacc.Bacc("TRN2", target_bir_lowering=False, debug=False,
                   num_devices=N_CORES)
    S_ext = nc.declare_dram_parameter(
        "sentence_embed", [B_CORE, T, D], fp32, isOutput=False)
    L_ext = nc.declare_dram_parameter("label_embed", [C, D], fp32, isOutput=False)
    W_ext = nc.declare_dram_parameter("conv_w", [K, C, F], fp32, isOutput=False)
    b_ext = nc.declare_dram_parameter("conv_b", [F], fp32, isOutput=False)
    out_ext = nc.declare_dram_parameter("out", [B_CORE, D], fp32, isOutput=True)

    with tile.TileContext(nc) as tc:
        with (
            tc.tile_pool(name="const", bufs=1) as cpool,
            tc.tile_pool(name="stage", bufs=5) as stage_pool,
            tc.tile_pool(name="small", bufs=4) as small_pool,
            tc.tile_pool(name="snat", bufs=14) as snat_pool,
            tc.tile_pool(name="tmpp", bufs=8) as tmp_pool,
            tc.tile_pool(name="sT", bufs=8) as sT_pool,
            tc.tile_pool(name="att", bufs=6) as att_pool,
            tc.tile_pool(name="scr", bufs=2) as scr_pool,
            tc.tile_pool(name="hsb", bufs=2) as hsb_pool,
            tc.tile_pool(name="ps", bufs=4, space="PSUM") as ps_pool,
            tc.tile_pool(name="hps", bufs=2, space="PSUM") as hps_pool,
        ):
            NCH = B_CORE * TCH
            s_nats = [None] * NCH
            s_Ts = [None] * NCH
            tmps = [None] * NCH
            atts = [None] * NCH
            att4s = [None] * (NCH // 4)

            def emit_load(g):
                bi, ch = divmod(g, TCH)
                t0 = ch * 128
                stage = stage_pool.tile([128, D], fp32, tag="stage")
                nc.sync.dma_start(stage[:], S_ext[bi, t0:t0 + 128, :])
                s_nat = snat_pool.tile([128, D], bf16, tag="snat")
                nc.vector.tensor_copy(s_nat[:], stage[:])  # f32 -> bf16 on DVE
                # out[p, dc, t] = s_nat[t, dc*128+p] -- verified on HW.
                # xbar transpose needs a CONTIGUOUS dest; DVE-copy into
                # per-chunk haloed tiles (strided dests are fine there).
                tmp = tmp_pool.tile([128, DC, 128], bf16, tag="sTtmp")
                nc.scalar.dma_start(tmp[:], s_nat[:], transpose=True)
                sT = sT_pool.tile([128, DC, 132], bf16, tag="sT")
                nc.vector.tensor_copy(sT[:, :, 2:130], tmp[:])
                if ch == 0:
                    nc.vector.memset(sT[:, :, 0:2], 0.0)
                else:
                    nc.vector.tensor_copy(sT[:, :, 0:2],
                                          tmps[g - 1][:, :, 126:128])
                    nc.vector.tensor_copy(s_Ts[g - 1][:, :, 130:132],
                                          tmp[:, :, 0:2])
                if ch == TCH - 1:
                    nc.vector.memset(sT[:, :, 130:132], 0.0)
                s_nats[g] = s_nat
                s_Ts[g] = sT
                tmps[g] = tmp

            h_pss = [None] * B_CORE
            conv_lasts = [None] * NCH

            # Prefetch the first S chunks NOW: their DMAs + DVE work
            # start at t=0 and overlap the whole W2 preparation phase.
            LEAD = 4
            for _g in range(min(LEAD, NCH)):
                emit_load(_g)

            # ---------------- Phase 0: constants -----------------
            if with_bias:
                # bias broadcast [128, F] via K=1 ones matmul
                ones_bf = cpool.tile([1, 128], bf16)
                nc.vector.memset(ones_bf[:], 1.0)
                b_f32 = cpool.tile([1, F], fp32)
                nc.sync.dma_start(b_f32[:], b_ext[:])
                b_bf = cpool.tile([1, F], bf16)
                nc.scalar.copy(b_bf[:], b_f32[:])
                bias_ps = ps_pool.tile([128, F], fp32, tag="ps")
                nc.tensor.matmul(bias_ps[:], lhsT=ones_bf[:], rhs=b_bf[:],
                                 start=True, stop=True)
                bias_sb = cpool.tile([128, F], fp32)
                nc.scalar.copy(bias_sb[:], bias_ps[:])

            # conv weights bf16 [c_in_chunk, (k, cc), f]; one HWDGE load
            # (40 SWDGE cast-DMAs cost ~1us Q7 emission each = 40us serial)
            w_view = W_ext.ap().rearrange("k (cc p) f -> p (k cc) f", p=128)
            w_sb = cpool.tile([128, K * CC, F], bf16)
            half = K * CC // 2
            for hf in range(2):
                w_stage = stage_pool.tile([128, half, F], fp32, tag="wstage", bufs=1)
                nc.sync.dma_start(w_stage[:], w_view[:, hf * half:(hf + 1) * half, :])
                for i in range(half):
                    nc.vector.tensor_copy(w_sb[:, hf * half + i, :], w_stage[:, i, :])

            # l2-normalized labels, bf16, layout [c_in_chunk, cc, d]
            l_norm = cpool.tile([128, CC, D], bf16)
            for cc in range(CC):
                l_f32 = stage_pool.tile([128, D], fp32, tag="lf32", bufs=2)
                nc.sync.dma_start(l_f32[:], L_ext[cc * 128:(cc + 1) * 128, :])
                sq = small_pool.tile([128, 1], fp32, tag="sq")
                sqscr = scr_pool.tile([128, D], fp32, tag="sqscr", bufs=1)
                nc.scalar.activation(sqscr[:], l_f32[:],
                                     mybir.ActivationFunctionType.Square,
                                     accum_out=sq[:])
                nc.vector.tensor_scalar_max(sq[:], sq[:], EPS)
                rt = small_pool.tile([128, 1], fp32, tag="rt")
                nc.scalar.sqrt(rt[:], sq[:])
                inv = small_pool.tile([128, 1], fp32, tag="inv")
                nc.vector.reciprocal(inv[:], rt[:])
                nc.vector.tensor_scalar_mul(l_norm[:, cc, :], l_f32[:], inv[:])

            # W2[k, d, f] = sum_c l_norm[c, d] w[k, c, f]; bf16 [d_in_chunk, (k, dc), f]
            w2_sb = cpool.tile([128, K * DC, F], bf16)
            for k in range(K):
                for dc in range(DC):
                    w2_ps = ps_pool.tile([128, F], fp32, tag="ps")
                    for cc in range(CC):
                        nc.tensor.matmul(
                            w2_ps[:],
                            lhsT=l_norm[:, cc, dc * 128:(dc + 1) * 128],
                            rhs=w_sb[:, k * CC + cc, :],
                            start=(cc == 0), stop=(cc == CC - 1))
                    nc.scalar.copy(w2_sb[:, k * DC + dc, :], w2_ps[:])

            # ---------------- Phase 1: main loop -----------------
            # Conv layout A: out[t, f] with S^T slices as stationary operand
            # (PE streams at ~2.0 GHz effective under sustained load, so
            # N=256 x 5120 MMs and N=512 x 2560 MMs cost the same; this
            # layout keeps the free-axis max over f, which DVE can do).
            def emit_compute(g):
                bi, ch = divmod(g, TCH)
                conv_ps = ps_pool.tile([128, F], fp32, tag="ps")
                mm = 0
                for k in range(K):
                    for dc in range(DC):
                        # lhsT[d, i] = S[t0+i+k-2, d]; +2 halo cancels -2
                        nc.tensor.matmul(
                            conv_ps[:],
                            lhsT=s_Ts[g][:, dc, k: k + 128],
                            rhs=w2_sb[:, k * DC + dc, :],
                            start=(mm == 0), stop=(mm == K * DC - 1))
                        mm += 1
                # att = relu(max_f(conv + b)); relu+cast on the idle GpSimd
                # so it never queues behind DVE copies
                att_f = att_pool.tile([128, 1], fp32, tag="attf")
                if with_bias:
                    scr = scr_pool.tile([128, F], fp32, tag="scr")
                    nc.vector.tensor_tensor(out=scr[:], in0=conv_ps[:],
                                            in1=bias_sb[:], op=ALU.add)
                    nc.vector.reduce_max(att_f[:], scr[:],
                                         axis=mybir.AxisListType.X)
                else:
                    nc.vector.reduce_max(att_f[:], conv_ps[:],
                                         axis=mybir.AxisListType.X)
                if ch % 4 == 0:
                    att4 = att_pool.tile([128, 4], bf16, tag="att")
                    att4s[g // 4] = att4
                nc.gpsimd.tensor_scalar_max(att4s[g // 4][:, ch % 4:ch % 4 + 1],
                                            att_f[:], 0.0)

            def emit_h4(grp):
                # one 8-MM H session per 4 chunks amortizes the PE
                # accumulation-group switch tax (~290ns/chunk otherwise)
                g0 = grp * 4
                bi, ch0 = divmod(g0, TCH)
                if ch0 == 0:
                    h_ps = hps_pool.tile([1, D], fp32, tag="hps")
                    h_pss[bi] = h_ps
                for c in range(4):
                    for j in range(2):
                        nc.tensor.matmul(
                            h_pss[bi][:, j * 512:(j + 1) * 512],
                            lhsT=att4s[grp][:, c:c + 1],
                            rhs=s_nats[g0 + c][:, j * 512:(j + 1) * 512],
                            start=(ch0 == 0 and c == 0),
                            stop=(ch0 + 4 == TCH and c == 3))
                if ch0 + 4 == TCH:
                    h_sb = hsb_pool.tile([1, D], fp32, tag="hsb")
                    nc.scalar.copy(h_sb[:], h_pss[bi][:])
                    nc.sync.dma_start(out_ext[bi, :], h_sb[:])

            # Flat global loop: the H lag and the load lookahead both cross
            # batch boundaries, so the pipeline never drains. Compute is
            # emitted BEFORE the iteration's load so reduce_max(g) sits in
            # front of copies(g+LEAD) on the in-order DVE queue.
            for g in range(NCH):
                emit_compute(g)
                if g % 4 == 3 and g >= 7:
                    emit_h4(g // 4 - 1)
                if g + LEAD < NCH:
                    emit_load(g + LEAD)
            emit_h4(NCH // 4 - 1)

    nc.compile()
    return nc


def _get_nc(with_bias=False):
    key = ("nc", bool(with_bias))
    if key not in _CACHE:
        _CACHE[key] = _build_nc(with_bias)
    return _CACHE[key]


def run_sharded(inputs, trace=False, tmpdir=None):
    """Run the SPMD kernel; returns (full_output [B, D], BassKernelResults)."""
    from concourse.bass_utils import run_bass_kernel_spmd

    bb_arr = np.asarray(inputs["conv_b"], dtype=np.float32)
    nc = _get_nc(with_bias=bool(np.any(bb_arr)))
    S = np.ascontiguousarray(np.asarray(inputs["sentence_embed"], dtype=np.float32))
    L = np.ascontiguousarray(np.asarray(inputs["label_embed"], dtype=np.float32))
    W = np.ascontiguousarray(np.asarray(inputs["conv_w"], dtype=np.float32))
    bb = np.ascontiguousarray(np.asarray(inputs["conv_b"], dtype=np.float32))
    in_maps = [
        {
            "sentence_embed": S[i * B_CORE:(i + 1) * B_CORE],
            "label_embed": L,
            "conv_w": W,
            "conv_b": bb,
        }
        for i in range(N_CORES)
    ]
    res = run_bass_kernel_spmd(nc, in_maps, core_ids=list(range(N_CORES)),
                               trace=trace, tmpdir=tmpdir)
    out = np.concatenate([res.results[i]["out"] for i in range(N_CORES)], axis=0)
    return out, res


def kernel(**inputs) -> np.ndarray:
    out, _ = run_sharded(inputs, trace=False)
    return out



# revision 16
# speedup vs baseline: 1.0308x; 1.0308x over previous
"""Trainium2 Bass kernel for nn_AttEncode (8-core data-parallel over batch).

Reference computation (B=64, T=2048, D=1024, C=1024, F=256, K=5):
    label_norm = l2_normalize(label_embed, axis=-1)          # [C, D]
    G          = einsum('btd,cd->btc', S, label_norm)        # [B, T, C]
    conv       = relu(conv1d_same(G, conv_w) + conv_b)       # [B, T, F]
    att_v      = max(conv, axis=-1)                          # [B, T]
    H          = einsum('btd,bt->bd', S, att_v)              # [B, D]

Key algebraic reduction: G is only consumed linearly by the conv, so fold the
label matmul into the conv weights once:
    W2[k, d, f] = sum_c label_norm[c, d] * conv_w[k, c, f]   # [K, D, F] tiny
    conv[t, f]  = sum_k sum_d S[t+k-2, d] * W2[k, d, f]
which removes the entire [B,T,C] einsum (45% of the FLOPs).

Per-core dataflow (8 batches/core, ~1.0 ms on silicon):
  - HWDGE-load S f32, convert to bf16 on ScalarE, then one xbar
    DMA-transpose per 128-token chunk gives S^T tiles in [d, t] layout
    (transpose needs a contiguous dest; DVE copies assemble 132-wide
    haloed per-chunk tiles for the SAME conv padding)
  - 40 bf16 matmuls per chunk accumulate conv[t0:t0+128, 0:256] in PSUM
    (stationary operand = S^T slice; PE streams ~2 GHz effective)
  - DVE reduce_max over F + relu/bf16-cast on GpSimd -> att column [128,1]
  - att feeds two skinny N=512 matmuls accumulating H[b, :] in PSUM,
    emitted 4 chunks behind the conv so the att chain never stalls PE
  - loads/transposes run 4 chunks ahead in a flat loop over all 128
    chunks, so neither the H lag nor the lookahead drains at batch
    boundaries
"""

import os
import numpy as np

B, T, D, C, F, K = 64, 2048, 1024, 1024, 256, 5
N_CORES = 8
B_CORE = B // N_CORES
EPS = 1e-12
TCH = T // 128  # t-chunks per batch
DC = D // 128   # d chunks
CC = C // 128   # c chunks

_CACHE = {}


def _build_nc(with_bias):
    import concourse.mybir as mybir
    import concourse.tile as tile
    from concourse import bacc

    fp32 = mybir.dt.float32
    bf16 = mybir.dt.bfloat16
    ALU = mybir.AluOpType

    nc = bacc.Bacc("TRN2", target_bir_lowering=False, debug=False,
                   num_devices=N_CORES)
    S_ext = nc.declare_dram_parameter(
        "sentence_embed", [B_CORE, T, D], fp32, isOutput=False)
    L_ext = nc.declare_dram_parameter("label_embed", [C, D], fp32, isOutput=False)
    W_ext = nc.declare_dram_parameter("conv_w", [K, C, F], fp32, isOutput=False)
    b_ext = nc.declare_dram_parameter("conv_b", [F], fp32, isOutput=False)
    out_ext = nc.declare_dram_parameter("out", [B_CORE, D], fp32, isOutput=True)

    with tile.TileContext(nc) as tc:
        with (
            tc.tile_pool(name="const", bufs=1) as cpool,
            tc.tile_pool(name="stage", bufs=7) as stage_pool,
            tc.tile_pool(name="small", bufs=4) as small_pool,
            tc.tile_pool(name="snat", bufs=15) as snat_pool,
            tc.tile_pool(name="tmpp", bufs=10) as tmp_pool,
            tc.tile_pool(name="sT", bufs=10) as sT_pool,
            tc.tile_pool(name="att", bufs=6) as att_pool,
            tc.tile_pool(name="scr", bufs=2) as scr_pool,
            tc.tile_pool(name="hsb", bufs=2) as hsb_pool,
            tc.tile_pool(name="ps", bufs=4, space="PSUM") as ps_pool,
            tc.tile_pool(name="hps", bufs=2, space="PSUM") as hps_pool,
        ):
            NCH = B_CORE * TCH
            s_nats = [None] * NCH
            s_Ts = [None] * NCH
            tmps = [None] * NCH
            atts = [None] * NCH
            att4s = [None] * (NCH // 4)

            def emit_load(g):
                bi, ch = divmod(g, TCH)
                t0 = ch * 128
                stage = stage_pool.tile([128, D], fp32, tag="stage")
                nc.sync.dma_start(stage[:], S_ext[bi, t0:t0 + 128, :])
                s_nat = snat_pool.tile([128, D], bf16, tag="snat")
                nc.vector.tensor_copy(s_nat[:], stage[:])  # f32 -> bf16 on DVE
                # out[p, dc, t] = s_nat[t, dc*128+p] -- verified on HW.
                # xbar transpose needs a CONTIGUOUS dest; DVE-copy into
                # per-chunk haloed tiles (strided dests are fine there).
                tmp = tmp_pool.tile([128, DC, 128], bf16, tag="sTtmp")
                nc.scalar.dma_start(tmp[:], s_nat[:], transpose=True)
                sT = sT_pool.tile([128, DC, 132], bf16, tag="sT")
                nc.vector.tensor_copy(sT[:, :, 2:130], tmp[:])
                if ch == 0:
                    nc.vector.memset(sT[:, :, 0:2], 0.0)
                else:
                    nc.vector.tensor_copy(sT[:, :, 0:2],
                                          tmps[g - 1][:, :, 126:128])
                    nc.vector.tensor_copy(s_Ts[g - 1][:, :, 130:132],
                                          tmp[:, :, 0:2])
                if ch == TCH - 1:
                    nc.vector.memset(sT[:, :, 130:132], 0.0)
                s_nats[g] = s_nat
                s_Ts[g] = sT
                tmps[g] = tmp

            h_pss = [None] * B_CORE
            conv_lasts = [None] * NCH

            # Prefetch the first S chunks NOW: their DMAs + DVE work
            # start at t=0 and overlap the whole W2 preparation phase.
            LEAD = 6
            for _g in range(min(LEAD, NCH)):
                emit_load(_g)

            # ---------------- Phase 0: constants -----------------
            if with_bias:
                # bias broadcast [128, F] via K=1 ones matmul
                ones_bf = cpool.tile([1, 128], bf16)
                nc.vector.memset(ones_bf[:], 1.0)
                b_f32 = cpool.tile([1, F], fp32)
                nc.sync.dma_start(b_f32[:], b_ext[:])
                b_bf = cpool.tile([1, F], bf16)
                nc.scalar.copy(b_bf[:], b_f32[:])
                bias_ps = ps_pool.tile([128, F], fp32, tag="ps")
                nc.tensor.matmul(bias_ps[:], lhsT=ones_bf[:], rhs=b_bf[:],
                                 start=True, stop=True)
                bias_sb = cpool.tile([128, F], fp32)
                nc.scalar.copy(bias_sb[:], bias_ps[:])

            # conv weights bf16 [c_in_chunk, (k, cc), f]; one HWDGE load
            # (40 SWDGE cast-DMAs cost ~1us Q7 emission each = 40us serial)
            w_view = W_ext.ap().rearrange("k (cc p) f -> p (k cc) f", p=128)
            w_sb = cpool.tile([128, K * CC, F], bf16)
            qtr = K * CC // 4
            for hf in range(4):
                w_stage = stage_pool.tile([128, qtr, F], fp32, tag="wstage", bufs=1)
                nc.sync.dma_start(w_stage[:], w_view[:, hf * qtr:(hf + 1) * qtr, :])
                for i in range(qtr):
                    nc.vector.tensor_copy(w_sb[:, hf * qtr + i, :], w_stage[:, i, :])

            # l2-normalized labels, bf16, layout [c_in_chunk, cc, d]
            l_norm = cpool.tile([128, CC, D], bf16)
            for cc in range(CC):
                l_f32 = stage_pool.tile([128, D], fp32, tag="lf32", bufs=2)
                nc.sync.dma_start(l_f32[:], L_ext[cc * 128:(cc + 1) * 128, :])
                sq = small_pool.tile([128, 1], fp32, tag="sq")
                sqscr = scr_pool.tile([128, D], fp32, tag="sqscr", bufs=1)
                nc.scalar.activation(sqscr[:], l_f32[:],
                                     mybir.ActivationFunctionType.Square,
                                     accum_out=sq[:])
                nc.vector.tensor_scalar_max(sq[:], sq[:], EPS)
                rt = small_pool.tile([128, 1], fp32, tag="rt")
                nc.scalar.sqrt(rt[:], sq[:])
                inv = small_pool.tile([128, 1], fp32, tag="inv")
                nc.vector.reciprocal(inv[:], rt[:])
                nc.vector.tensor_scalar_mul(l_norm[:, cc, :], l_f32[:], inv[:])

            # W2[k, d, f] = sum_c l_norm[c, d] w[k, c, f]; bf16 [d_in_chunk, (k, dc), f]
            w2_sb = cpool.tile([128, K * DC, F], bf16)
            for k in range(K):
                for dc in range(DC):
                    w2_ps = ps_pool.tile([128, F], fp32, tag="ps")
                    for cc in range(CC):
                        nc.tensor.matmul(
                            w2_ps[:],
                            lhsT=l_norm[:, cc, dc * 128:(dc + 1) * 128],
                            rhs=w_sb[:, k * CC + cc, :],
                            start=(cc == 0), stop=(cc == CC - 1))
                    nc.scalar.copy(w2_sb[:, k * DC + dc, :], w2_ps[:])

            # ---------------- Phase 1: main loop -----------------
            # Conv layout A: out[t, f] with S^T slices as stationary operand
            # (PE streams at ~2.0 GHz effective under sustained load, so
            # N=256 x 5120 MMs and N=512 x 2560 MMs cost the same; this
            # layout keeps the free-axis max over f, which DVE can do).
            def emit_compute(g):
                bi, ch = divmod(g, TCH)
                conv_ps = ps_pool.tile([128, F], fp32, tag="ps")
                mm = 0
                for k in range(K):
                    for dc in range(DC):
                        # lhsT[d, i] = S[t0+i+k-2, d]; +2 halo cancels -2
                        nc.tensor.matmul(
                            conv_ps[:],
                            lhsT=s_Ts[g][:, dc, k: k + 128],
                            rhs=w2_sb[:, k * DC + dc, :],
                            start=(mm == 0), stop=(mm == K * DC - 1))
                        mm += 1
                # att = relu(max_f(conv + b)); relu+cast on the idle GpSimd
                # so it never queues behind DVE copies
                att_f = att_pool.tile([128, 1], fp32, tag="attf")
                if with_bias:
                    scr = scr_pool.tile([128, F], fp32, tag="scr")
                    nc.vector.tensor_tensor(out=scr[:], in0=conv_ps[:],
                                            in1=bias_sb[:], op=ALU.add)
                    nc.vector.reduce_max(att_f[:], scr[:],
                                         axis=mybir.AxisListType.X)
                else:
                    nc.vector.reduce_max(att_f[:], conv_ps[:],
                                         axis=mybir.AxisListType.X)
                if ch % 4 == 0:
                    att4 = att_pool.tile([128, 4], bf16, tag="att")
                    att4s[g // 4] = att4
                nc.gpsimd.tensor_scalar_max(att4s[g // 4][:, ch % 4:ch % 4 + 1],
                                            att_f[:], 0.0)

            def emit_h4(grp):
                # one 8-MM H session per 4 chunks amortizes the PE
                # accumulation-group switch tax (~290ns/chunk otherwise)
                g0 = grp * 4
                bi, ch0 = divmod(g0, TCH)
                if ch0 == 0:
                    h_ps = hps_pool.tile([1, D], fp32, tag="hps")
                    h_pss[bi] = h_ps
                for c in range(4):
                    for j in range(2):
                        nc.tensor.matmul(
                            h_pss[bi][:, j * 512:(j + 1) * 512],
                            lhsT=att4s[grp][:, c:c + 1],
                            rhs=s_nats[g0 + c][:, j * 512:(j + 1) * 512],
                            start=(ch0 == 0 and c == 0),
                            stop=(ch0 + 4 == TCH and c == 3))
                if ch0 + 4 == TCH:
                    h_sb = hsb_pool.tile([1, D], fp32, tag="hsb")
                    nc.vector.tensor_copy(h_sb[:], h_pss[bi][:])
                    nc.sync.dma_start(out_ext[bi, :], h_sb[:])

            # Flat global loop: the H lag and the load lookahead both cross
            # batch boundaries, so the pipeline never drains. Compute is
            # emitted BEFORE the iteration's load so reduce_max(g) sits in
            # front of copies(g+LEAD) on the in-order DVE queue.
            for g in range(NCH):
                emit_compute(g)
                if g % 4 == 3 and g >= 7:
                    emit_h4(g // 4 - 1)
                if g + LEAD < NCH:
                    emit_load(g + LEAD)
            emit_h4(NCH // 4 - 1)

    nc.compile()
    return nc


def _get_nc(with_bias=False):
    key = ("nc", bool(with_bias))
    if key not in _CACHE:
        _CACHE[key] = _build_nc(with_bias)
    return _CACHE[key]


def run_sharded(inputs, trace=False, tmpdir=None):
    """Run the SPMD kernel; returns (full_output [B, D], BassKernelResults)."""
    from concourse.bass_utils import run_bass_kernel_spmd

    bb_arr = np.asarray(inputs["conv_b"], dtype=np.float32)
    nc = _get_nc(with_bias=bool(np.any(bb_arr)))
    S = np.ascontiguousarray(np.asarray(inputs["sentence_embed"], dtype=np.float32))
    L = np.ascontiguousarray(np.asarray(inputs["label_embed"], dtype=np.float32))
    W = np.ascontiguousarray(np.asarray(inputs["conv_w"], dtype=np.float32))
    bb = np.ascontiguousarray(np.asarray(inputs["conv_b"], dtype=np.float32))
    in_maps = [
        {
            "sentence_embed": S[i * B_CORE:(i + 1) * B_CORE],
            "label_embed": L,
            "conv_w": W,
            "conv_b": bb,
        }
        for i in range(N_CORES)
    ]
    res = run_bass_kernel_spmd(nc, in_maps, core_ids=list(range(N_CORES)),
                               trace=trace, tmpdir=tmpdir)
    out = np.concatenate([res.results[i]["out"] for i in range(N_CORES)], axis=0)
    return out, res


def kernel(**inputs) -> np.ndarray:
    out, _ = run_sharded(inputs, trace=False)
    return out

